# revision 23
# baseline (speedup 1.0000x reference)
"""Trainium2 Bass kernel for nn_MultiHeadAttention_55336358642102.

Strategy: data-parallel over the 8 equal-length sentences (B=8) — one
sentence per NeuronCore, no collectives. Each core computes, for its
[L=1024, D=1024] slice:
  - Q^T/K^T per head via weight-stationary matmuls (heads packed in pairs
    so the PE runs with M=128) on a host-pretransposed X^T; V in natural
    [token, dv] layout directly (lhsT = X^T chunks).
  - attention in "transposed score" space: S^T = K^T-chunks.T @ Q^T so the
    softmaxed probabilities come out with keys on partitions, which is the
    exact layout the P@V matmul needs (lhsT = V-natural chunks).
  - softmax without max-subtraction (logits are ~N(0, 0.15) here — exact
    softmax is shift-invariant so this matches the reference); the
    denominator comes from an all-ones-lhsT matmul over exp(S^T), which
    also replicates it across psum partitions for the normalize step.
  - output projection with the per-head halves packed into two [512, L]
    operands (O1T/O2T) matching w_proj1/w_proj2 row order, then residual +
    unbiased-std layernorm in fp32.

Matmul operands are bf16 (full PE rate); accumulation, residual and
layernorm are fp32. All DRAM inputs are pre-arranged partition-major so
every load is one 2D DMA. Partition-range routing (head halves into
packed operands) is done with SBUF->SBUF DMAs, which unlike the compute
engines can shift partitions.
"""

import sys

import ml_dtypes
import numpy as np

if "/opt/trn_rl_repo" not in sys.path:
    sys.path.insert(0, "/opt/trn_rl_repo")

import concourse.bass as bass
import concourse.mybir as mybir
import concourse.tile as tile
from concourse import bacc
from concourse.bass import ds
from concourse.bass_utils import run_bass_kernel_spmd

P = 128
L = 1024            # rows per core (= max_len; one sentence per core)
DM = 1024           # d_model
DC, DP = 768, 256   # content / positional feature split
NKC, NKP = DC // P, DP // P     # 6, 2 feature chunks
NPAIR = 4
NCORES = 8
INV_TEMPER = 1.0 / 32.0         # 1/sqrt(DM)
EPS = 1e-3
F32 = mybir.dt.float32
BF16 = mybir.dt.bfloat16
AF = mybir.ActivationFunctionType
ALU = mybir.AluOpType
BF16NP = ml_dtypes.bfloat16


def build_nc(apply_ln: bool) -> bass.Bass:
    nc = bacc.Bacc(None, target_bir_lowering=False)

    # all inputs are pre-arranged on the host to be partition-major and
    # contiguous per partition, so every load is a single 2D DMA pattern
    xt = nc.dram_tensor("xt", [P, DM // P, L], BF16, kind="ExternalInput")
    xr = nc.dram_tensor("xr", [L, DM], F32, kind="ExternalInput")
    wc_d = nc.dram_tensor("wc", [P, NPAIR, NKC, 3, P], BF16, kind="ExternalInput")
    wp_d = nc.dram_tensor("wp", [P, NPAIR, NKP, 3, P], BF16, kind="ExternalInput")
    w1_d = nc.dram_tensor("w1", [P, 4, DC], BF16, kind="ExternalInput")
    w2_d = nc.dram_tensor("w2", [P, 4, DP], BF16, kind="ExternalInput")
    if apply_ln:
        lna_d = nc.dram_tensor("lna", [1, DM], F32, kind="ExternalInput")
        lnb_d = nc.dram_tensor("lnb", [1, DM], F32, kind="ExternalInput")
    out_d = nc.dram_tensor("out", [L, DM], F32, kind="ExternalOutput")

    with tile.TileContext(nc) as tc:
        with (
            tc.tile_pool(name="sing", bufs=1) as sing,
            tc.tile_pool(name="wpool", bufs=2) as wpool,
            tc.tile_pool(name="qkt", bufs=2) as qkt,
            tc.tile_pool(name="epool", bufs=4) as epool,
            tc.tile_pool(name="dpool", bufs=4) as dpool,
            tc.tile_pool(name="stg", bufs=6) as stg,
            tc.tile_pool(name="zpool", bufs=2) as zpool,
            tc.tile_pool(name="xpool", bufs=2) as xpool,
            tc.tile_pool(name="stat", bufs=3) as stat,
            tc.tile_pool(name="ps_mm", bufs=3, space="PSUM") as ps_mm,
            tc.tile_pool(name="ps_pv", bufs=3, space="PSUM") as ps_pv,
            tc.tile_pool(name="ps_d", bufs=2, space="PSUM") as ps_d,
        ):
            # ---- resident constants -------------------------------------
            # X^T feature chunks as separate tiles: fine-grained DMA deps so
            # the first QKV matmuls start as soon as their chunk lands
            XTs = []
            for o in range(DM // P):
                xto = sing.tile([P, L], BF16, name=f"xt{o}")
                nc.gpsimd.dma_start(xto, xt[:, o])
                XTs.append(xto)

            ones = sing.tile([P, P], BF16)
            nc.vector.memset(ones, 1.0)

            if apply_ln:
                LNA = sing.tile([1, DM], F32)
                nc.sync.dma_start(LNA, lna_d[:])
                LNB = sing.tile([1, DM], F32)
                nc.sync.dma_start(LNB, lnb_d[:])

            O1T = sing.tile([P, 4, L], BF16)   # packed (head, dv<64) rows x t
            O2T = sing.tile([P, 4, L], BF16)

            lo = slice(0, 64)
            hi = slice(64, 128)

            for j in range(NPAIR):
                # ---- Phase A: QKV for head pair (2j, 2j+1) --------------
                wc = wpool.tile([P, NKC, 3, P], BF16, tag="wc")
                nc.sync.dma_start(wc, wc_d[:, j])
                wp = wpool.tile([P, NKP, 3, P], BF16, tag="wp")
                nc.sync.dma_start(wp, wp_d[:, j])

                # per-head layouts, uniform [content | pos] ordering:
                #   QT/KT [p=dk, head-in-pair, t]
                QT = qkt.tile([P, 2, L], BF16, tag="qt")
                KT = qkt.tile([P, 2, L], BF16, tag="kt")
                V = qkt.tile([P, 8, 2, P], BF16, tag="v")

                for s, DST in ((0, QT), (1, KT)):
                    for half in range(2):
                        hs = ds(half * 512, 512)
                        pc = ps_mm.tile([P, 512], F32, tag="mm")
                        for kc in range(NKC):
                            nc.tensor.matmul(
                                pc, wc[:, kc, s, :], XTs[kc][:, hs],
                                start=(kc == 0), stop=(kc == NKC - 1))
                        pp = ps_mm.tile([P, 512], F32, tag="mm")
                        for kc in range(NKP):
                            nc.tensor.matmul(
                                pp, wp[:, kc, s, :], XTs[NKC + kc][:, hs],
                                start=(kc == 0), stop=(kc == NKP - 1))
                        # shift-free halves go straight from psum to the
                        # packed layout; the other halves stage then DMA
                        # (only DMA can shift partition ranges)
                        nc.vector.tensor_copy(DST[lo, 0, hs], pc[lo])
                        nc.vector.tensor_copy(DST[hi, 0, hs], pp[hi])
                        sc = stg.tile([P, 512], BF16, tag="sc")
                        nc.vector.tensor_copy(sc[hi], pc[hi])
                        sp = stg.tile([P, 512], BF16, tag="sp")
                        nc.vector.tensor_copy(sp[lo], pp[lo])
                        nc.gpsimd.dma_start(DST[lo, 1, hs], sc[hi])
                        nc.gpsimd.dma_start(DST[hi, 1, hs], sp[lo])

                # V natural: out[token, dv] = sum_f X^T[f, token] * Wv[f, dv]
                for rc in range(8):
                    rsl = ds(rc * P, P)
                    pv_n = ps_mm.tile([P, 512], F32, tag="mm")
                    for kc in range(NKC):
                        nc.tensor.matmul(
                            pv_n[:, 0:128], XTs[kc][:, rsl], wc[:, kc, 2, :],
                            start=(kc == 0), stop=(kc == NKC - 1))
                    for kc in range(NKP):
                        nc.tensor.matmul(
                            pv_n[:, 128:256], XTs[NKC + kc][:, rsl],
                            wp[:, kc, 2, :],
                            start=(kc == 0), stop=(kc == NKP - 1))
                    # psum cols [h c | h' c | h p | h' p] -> per-head
                    # contiguous [cont|pos] blocks via a strided source AP
                    nc.vector.tensor_copy(
                        V[:, rc],
                        pv_n[:, 0:256].rearrange(
                            "p (half head e) -> p head half e",
                            half=2, head=2))

                # ---- Phase B: attention for the two heads ---------------
                for hh in range(2):
                    vb = V[:, :, hh, :]   # [p, chunk, dv]
                    for half in range(2):
                        hs = ds(half * 512, 512)
                        pv = ps_pv.tile([P, 512], F32, tag="pv")
                        dd = ps_d.tile([P, 512], F32, tag="d")
                        for c in range(8):
                            csl = ds(c * P, P)
                            pss = ps_mm.tile([P, 512], F32, tag="mm")
                            nc.tensor.matmul(
                                pss, KT[:, hh, csl],
                                QT[:, hh, hs], start=True, stop=True)
                            e = epool.tile([P, 512], BF16, tag="e")
                            nc.scalar.activation(e, pss, AF.Exp,
                                                 scale=INV_TEMPER)
                            nc.tensor.matmul(
                                pv, vb[:, c], e,
                                start=(c == 0), stop=(c == 7))
                            nc.tensor.matmul(
                                dd, ones, e,
                                start=(c == 0), stop=(c == 7))

                        # normalize by 1/d (already replicated across psum
                        # partitions by the all-ones lhsT), stage, route
                        rd = dpool.tile([P, 512], F32, tag="rd")
                        nc.vector.reciprocal_approx_fast(rd, dd)  # psum->sbuf
                        no = stg.tile([P, 512], BF16, tag="no")
                        nc.vector.tensor_mul(no, pv, rd)
                        # psum rows [o1 | o2] for every head; route to the
                        # packed operands
                        if hh == 0:
                            nc.gpsimd.dma_start(O1T[lo, j, hs], no[lo])
                            nc.gpsimd.dma_start(O2T[lo, j, hs], no[hi])
                        else:
                            nc.gpsimd.dma_start(O1T[hi, j, hs], no[lo])
                            nc.gpsimd.dma_start(O2T[hi, j, hs], no[hi])

            # ---- Phase C: output projection + residual + layernorm ------
            W1 = sing.tile([P, 4, DC], BF16)
            nc.sync.dma_start(W1, w1_d[:])
            W2 = sing.tile([P, 4, DP], BF16)
            nc.sync.dma_start(W2, w2_d[:])
            for t in range(L // P):
                tsl = ds(t * P, P)
                poa = ps_pv.tile([P, 512], F32, tag="pv")   # o1[:, 0:512]
                pob = ps_d.tile([P, 512], F32, tag="d")     # o1[:,512:768] | o2
                for kc in range(4):
                    nc.tensor.matmul(poa, O1T[:, kc, tsl],
                                     W1[:, kc, 0:512],
                                     start=kc == 0, stop=kc == 3)
                for kc in range(4):
                    nc.tensor.matmul(pob[:, 0:256], O1T[:, kc, tsl],
                                     W1[:, kc, 512:768],
                                     start=kc == 0, stop=kc == 3)
                for kc in range(4):
                    nc.tensor.matmul(pob[:, 256:512], O2T[:, kc, tsl],
                                     W2[:, kc, :],
                                     start=kc == 0, stop=kc == 3)

                xts = xpool.tile([P, DM], F32, tag="x")
                nc.sync.dma_start(xts, xr[tsl, :])
                z = zpool.tile([P, DM], F32, tag="z")
                nc.vector.tensor_add(z[:, 0:512], poa, xts[:, 0:512])
                nc.vector.tensor_add(z[:, 512:1024], pob, xts[:, 512:1024])

                stats = stat.tile([P, 2, 6], F32, tag="st")
                nc.vector.bn_stats(stats[:, 0], z[:, 0:512])
                nc.vector.bn_stats(stats[:, 1], z[:, 512:1024])
                mv = stat.tile([P, 2], F32, tag="mv")
                nc.vector.bn_aggr(mv, stats)
                sig = stat.tile([P, 1], F32, tag="sig")
                # unbiased std: sqrt(var * n/(n-1)), then +eps, then 1/x
                nc.scalar.activation(sig, mv[:, 1:2], AF.Sqrt,
                                     scale=float(DM) / (DM - 1))
                nc.vector.tensor_scalar_add(sig, sig, EPS)
                nc.vector.reciprocal_approx_fast(sig, sig)
                nc.vector.tensor_scalar(z, z, mv[:, 0:1], sig,
                                        ALU.subtract, ALU.mult)
                if apply_ln:
                    nc.vector.tensor_mul(z, z, LNA.to_broadcast((P, DM)))
                    nc.vector.tensor_add(z, z, LNB.to_broadcast((P, DM)))
                nc.sync.dma_start(out_d[tsl, :], z)

    nc.finalize()
    return nc


def _part_major(a, p=P):
    """[K*p, ...rest] -> [p, K, ...rest] contiguous (partition-major)."""
    k = a.shape[0] // p
    return np.ascontiguousarray(
        a.reshape((k, p) + a.shape[1:]).swapaxes(0, 1))


def _prep(inp, w_qs1, w_ks1, w_vs1, w_qs2, w_ks2, w_vs2, w_proj1, w_proj2):
    wc = np.empty((NPAIR, DC, 3, P), BF16NP)
    wp = np.empty((NPAIR, DP, 3, P), BF16NP)
    for j in range(NPAIR):
        for s, (wa, wb) in enumerate(((w_qs1, w_qs2), (w_ks1, w_ks2),
                                      (w_vs1, w_vs2))):
            wc[j, :, s, 0:64] = wa[2 * j]
            wc[j, :, s, 64:128] = wa[2 * j + 1]
            if s < 2:   # pos pair swapped for q/k (split-K row groups)
                wp[j, :, s, 0:64] = wb[2 * j + 1]
                wp[j, :, s, 64:128] = wb[2 * j]
            else:       # v keeps natural order
                wp[j, :, s, 0:64] = wb[2 * j]
                wp[j, :, s, 64:128] = wb[2 * j + 1]
    # -> [P, NPAIR, NK, 3, P] partition-major
    wc = np.ascontiguousarray(
        wc.reshape(NPAIR, NKC, P, 3, P).transpose(2, 0, 1, 3, 4))
    wp = np.ascontiguousarray(
        wp.reshape(NPAIR, NKP, P, 3, P).transpose(2, 0, 1, 3, 4))
    w1 = _part_major(np.asarray(w_proj1, np.float32).astype(BF16NP))
    w2 = _part_major(np.asarray(w_proj2, np.float32).astype(BF16NP))

    x = np.ascontiguousarray(np.asarray(inp, np.float32)).reshape(NCORES, L, DM)
    xts = [_part_major(x[b].T.astype(BF16NP)) for b in range(NCORES)]
    return x, xts, wc, wp, w1, w2


_NC_CACHE = {}


def _get_nc(apply_ln):
    if apply_ln not in _NC_CACHE:
        _NC_CACHE[apply_ln] = build_nc(apply_ln)
    return _NC_CACHE[apply_ln]


def kernel(inp, w_qs1, w_ks1, w_vs1, w_qs2, w_ks2, w_vs2, w_proj1, w_proj2,
           ln_a, ln_b, batch_size, max_len, _trace=False):
    inp = np.asarray(inp, np.float32)
    assert int(batch_size) == NCORES and int(max_len) == L
    assert inp.shape == (NCORES * L, DM)

    ln_a = np.asarray(ln_a, np.float32).reshape(-1)
    ln_b = np.asarray(ln_b, np.float32).reshape(-1)
    apply_ln = not (np.all(ln_a == 1.0) and np.all(ln_b == 0.0))

    x, xts, wc, wp, w1, w2 = _prep(
        inp, np.asarray(w_qs1, np.float32), np.asarray(w_ks1, np.float32),
        np.asarray(w_vs1, np.float32), np.asarray(w_qs2, np.float32),
        np.asarray(w_ks2, np.float32), np.asarray(w_vs2, np.float32),
        np.asarray(w_proj1, np.float32), np.asarray(w_proj2, np.float32))

    nc = _get_nc(apply_ln)

    in_maps = []
    for b in range(NCORES):
        m = dict(xt=xts[b], xr=np.ascontiguousarray(x[b]),
                 wc=wc, wp=wp, w1=w1, w2=w2)
        if apply_ln:
            m["lna"] = ln_a.reshape(1, DM)
            m["lnb"] = ln_b.reshape(1, DM)
        in_maps.append(m)

    res = run_bass_kernel_spmd(nc, in_maps, list(range(NCORES)), trace=_trace)
    out = np.concatenate([res.results[b]["out"] for b in range(NCORES)], 0)
    if _trace:
        return out, res
    return out
